# revision 21
# baseline (speedup 1.0000x reference)
"""Trainium2 Bass kernel for CausalSelfAttention with sliding-window + sink mask.

Sharding: 8 cores = (batch 2) x (sequence chunks of 512). Each core computes
QKV (+RoPE) for its 512 queries and for a kv range [4 sink | 256 halo |
512 own | 124 zero-pad] = 896 positions, runs banded attention in a
scores-transposed [k, q] layout (7 key-chunks of 128 with fixed q-windows,
multiplicative post-exp masking, denominator via a ones-column in V), then
projects with w_proj emitting a transposed [C, 512] output that the host
re-transposes and concatenates.

All matmuls run as float32r (full-rate fp32 path of the PE).
"""

import numpy as np

B, T, C, NH, HD = 2, 2048, 1024, 16, 64
WIN, SINK = 256, 4
CH = 512          # queries per core
KV = 896          # 512 own + 256 halo + 4 sink + 124 pad
NCORES = 8
W_C = [384, 512, 256, 256, 128, 256, 512]
OFF_C = [0, 0, 256, 256, 0, 0, 0]
MOFF = np.concatenate([[0], np.cumsum(W_C)]).astype(int)  # mask col offsets
MTOT = int(MOFF[-1])  # 1924

_cache = {}


def _build_nc():
    import concourse.bacc as bacc
    import concourse.mybir as mybir
    import concourse.tile as tile

    f32 = mybir.dt.float32
    f32r = mybir.dt.float32r
    AF = mybir.ActivationFunctionType

    nc = bacc.Bacc("TRN2", target_bir_lowering=False, debug=False,
                   num_devices=NCORES)

    xT = nc.dram_tensor("xT", [C, KV], f32r, kind="ExternalInput").ap()
    wqs = nc.dram_tensor("wqs", [C, C], f32r, kind="ExternalInput").ap()
    wks = nc.dram_tensor("wks", [C, C], f32r, kind="ExternalInput").ap()
    wv = nc.dram_tensor("wv", [C, C], f32r, kind="ExternalInput").ap()
    wps = nc.dram_tensor("wps", [C, C], f32r, kind="ExternalInput").ap()
    cos_q = nc.dram_tensor("cos_q", [128, CH], f32, kind="ExternalInput").ap()
    sin_q = nc.dram_tensor("sin_q", [128, CH], f32, kind="ExternalInput").ap()
    cos_k = nc.dram_tensor("cos_k", [128, KV], f32, kind="ExternalInput").ap()
    sin_k = nc.dram_tensor("sin_k", [128, KV], f32, kind="ExternalInput").ap()
    masks = nc.dram_tensor("masks", [128, MTOT], f32, kind="ExternalInput").ap()
    p2d = nc.dram_tensor("p2", [128, 128], f32r, kind="ExternalInput").ap()
    rseld = nc.dram_tensor("rsel", [16, C], f32r, kind="ExternalInput").ap()
    onesd = nc.dram_tensor("ones", [128, 16], f32, kind="ExternalInput").ap()
    outT = nc.dram_tensor("outT", [C, CH], f32, kind="ExternalOutput").ap()

    KSEG = [(0, 512), (512, 384)]  # kv free-dim segments (psum bank limit)

    with tile.TileContext(nc) as tc:
        with (
            tc.tile_pool(name="pers", bufs=1) as pers,
            tc.tile_pool(name="wsl", bufs=2) as wsl,
            tc.tile_pool(name="big", bufs=8) as big,     # wv chunks then praw/outT
            tc.tile_pool(name="qk", bufs=2) as qkp,
            tc.tile_pool(name="tmp", bufs=2) as tmp,
            tc.tile_pool(name="yts", bufs=1) as ytsp,
            tc.tile_pool(name="ptp", bufs=14) as ptp,
            tc.tile_pool(name="sm", bufs=2) as smp,
            tc.tile_pool(name="psmm", bufs=2, space="PSUM") as psmm,
            tc.tile_pool(name="pssc", bufs=4, space="PSUM") as pssc,
            tc.tile_pool(name="psyt", bufs=2, space="PSUM") as psyt,
        ):
            # ---------- persistent loads ----------
            xa, xb = [], []
            for i in range(8):
                t = pers.tile([128, 512], f32r, tag=f"xa{i}", name=f"xa{i}")
                nc.sync.dma_start(t[:], xT[i * 128:(i + 1) * 128, 0:512])
                xa.append(t)
                t = pers.tile([128, 384], f32r, tag=f"xb{i}", name=f"xb{i}")
                nc.sync.dma_start(t[:], xT[i * 128:(i + 1) * 128, 512:896])
                xb.append(t)
            tcos_q = pers.tile([128, CH], f32, tag="cos_q")
            nc.sync.dma_start(tcos_q[:], cos_q[:])
            tsin_q = pers.tile([128, CH], f32, tag="sin_q")
            nc.sync.dma_start(tsin_q[:], sin_q[:])
            tcos_k = pers.tile([128, KV], f32, tag="cos_k")
            nc.sync.dma_start(tcos_k[:], cos_k[:])
            tsin_k = pers.tile([128, KV], f32, tag="sin_k")
            nc.sync.dma_start(tsin_k[:], sin_k[:])
            tmask = pers.tile([128, MTOT], f32, tag="mask")
            nc.sync.dma_start(tmask[:], masks[:])
            tp2 = pers.tile([128, 128], f32r, tag="p2")
            nc.sync.dma_start(tp2[:], p2d[:])
            trsel = pers.tile([16, C], f32r, tag="rsel")
            nc.sync.dma_start(trsel[:], rseld[:])
            tones = pers.tile([128, 16], f32, tag="ones")
            nc.sync.dma_start(tones[:], onesd[:])

            # ---------- V = xT.T @ wv in [k, d] layout with ones columns ----------
            wvc = []
            for kc in range(8):
                t = big.tile([128, 1024], f32r, tag="big", name=f"wvc{kc}")
                nc.sync.dma_start(t[:], wv[kc * 128:(kc + 1) * 128, :])
                wvc.append(t)
            v_sb = []
            for tt in range(7):
                vt = pers.tile([128, 1040], f32r, tag=f"v{tt}", name=f"v{tt}")
                vr = vt.rearrange("p (h e) -> p h e", e=65)
                pv = [psmm.tile([128, 512], f32, tag="mm", name=f"pv{tt}_{i}")
                      for i in range(2)]
                for kc in range(8):
                    xsl = (xa[kc][:, tt * 128:(tt + 1) * 128] if tt < 4 else
                           xb[kc][:, (tt - 4) * 128:(tt - 3) * 128])
                    for dh in range(2):
                        nc.tensor.matmul(
                            pv[dh][:], xsl,
                            wvc[kc][:, dh * 512:(dh + 1) * 512],
                            start=(kc == 0), stop=(kc == 7),
                        )
                for dh in range(2):
                    nc.scalar.copy(
                        vr[:, dh * 8:(dh + 1) * 8, 0:64],
                        pv[dh][:].rearrange("p (h e) -> p h e", e=64),
                    )
                nc.scalar.copy(vr[:, :, 64:65],
                               tones[:].rearrange("p (h o) -> p h o", o=1))
                v_sb.append(vt)

            d16 = smp.tile([16, 512], f32, tag="d16")
            AVORD = [1, 6, 0, 5, 4, 2, 3]

            def qkv_rope(hp):
                # qT raw
                wq_sl = wsl.tile([128, 1024], f32r, tag="wslab",
                                 name=f"wq{hp}")
                nc.sync.dma_start(wq_sl[:], wqs[hp * 128:(hp + 1) * 128, :])
                pq = psmm.tile([128, 512], f32, tag="mm", name=f"pq{hp}")
                for kc in range(8):
                    nc.tensor.matmul(
                        pq[:], wq_sl[:, kc * 128:(kc + 1) * 128],
                        xa[kc][:],
                        start=(kc == 0), stop=(kc == 7),
                    )
                qraw = tmp.tile([128, CH], f32r, tag="qraw", name=f"qraw{hp}")
                nc.scalar.copy(qraw[:], pq[:])

                # kT raw (segments share each weight load)
                wk_sl = wsl.tile([128, 1024], f32r, tag="wslab",
                                 name=f"wk{hp}")
                nc.sync.dma_start(wk_sl[:], wks[hp * 128:(hp + 1) * 128, :])
                kraw = tmp.tile([128, KV], f32r, tag="kraw", name=f"kraw{hp}")
                pk = [psmm.tile([128, 512], f32, tag="mm", name=f"pk{hp}_{i}")
                      for i in range(2)]
                for kc in range(8):
                    for si, (s0, sw) in enumerate(KSEG):
                        rhs = xa[kc][:] if si == 0 else xb[kc][:]
                        nc.tensor.matmul(
                            pk[si][:, 0:sw], wk_sl[:, kc * 128:(kc + 1) * 128],
                            rhs, start=(kc == 0), stop=(kc == 7),
                        )
                for si, (s0, sw) in enumerate(KSEG):
                    nc.scalar.copy(kraw[:, s0:s0 + sw], pk[si][:, 0:sw])

                # rope
                qT = qkp.tile([128, CH], f32r, tag="qT", name=f"qT{hp}")
                prot = psmm.tile([128, 512], f32, tag="mm", name=f"prot{hp}")
                nc.tensor.matmul(prot[:], tp2[:], qraw[:], start=True, stop=True)
                t2 = tmp.tile([128, CH], f32, tag="t2", name=f"t2q{hp}")
                nc.vector.tensor_mul(t2[:], prot[:], tsin_q[:])
                nc.vector.tensor_mul(qraw[:], qraw[:], tcos_q[:])
                nc.vector.tensor_add(qT[:], qraw[:], t2[:])

                kT = qkp.tile([128, KV], f32r, tag="kT", name=f"kT{hp}")
                for si, (s0, sw) in enumerate(KSEG):
                    prk = psmm.tile([128, 512], f32, tag="mm",
                                    name=f"prk{hp}_{si}")
                    nc.tensor.matmul(prk[:, 0:sw], tp2[:],
                                     kraw[:, s0:s0 + sw], start=True, stop=True)
                    t2k = tmp.tile([128, 512], f32, tag="t2",
                                   name=f"t2k{hp}_{si}")
                    nc.vector.tensor_mul(t2k[:, 0:sw], prk[:, 0:sw],
                                         tsin_k[:, s0:s0 + sw])
                    nc.vector.tensor_mul(kraw[:, s0:s0 + sw],
                                         kraw[:, s0:s0 + sw],
                                         tcos_k[:, s0:s0 + sw])
                    nc.vector.tensor_add(kT[:, s0:s0 + sw],
                                         kraw[:, s0:s0 + sw], t2k[:, 0:sw])
                return qT, kT

            def sc_block(hp, qT, kT):
                # scoresT matmuls issued as adjacent row-tile pairs (K=64 at
                # partition bases 0/64 -> concurrent in the PE array), then
                # exp (psum->sbuf, fused 1/sqrt(hd) scale) and mask multiply.
                pts = {}
                for c in range(7):
                    w, off = W_C[c], OFF_C[c]
                    scs = []
                    for half in range(2):
                        dsl = slice(half * 64, half * 64 + 64)
                        sc = pssc.tile([128, 512], f32, tag="sc",
                                       name=f"sc{hp}_{c}_{half}")
                        nc.tensor.matmul(
                            sc[:, 0:w], kT[dsl, c * 128:(c + 1) * 128],
                            qT[dsl, off:off + w], start=True, stop=True,
                        )
                        scs.append(sc)
                    for half in range(2):
                        w, off = W_C[c], OFF_C[c]
                        praw = big.tile([128, 512], f32r, tag="big",
                                        name=f"praw{hp}_{c}_{half}")
                        nc.scalar.activation(praw[:, 0:w], scs[half][:, 0:w],
                                             AF.Exp, scale=0.125)
                        pt = ptp.tile([128, 512], f32r, tag="pt",
                                      name=f"pt{hp}_{c}_{half}")
                        nc.vector.tensor_mul(
                            pt[:, 0:w], praw[:, 0:w],
                            tmask[:, MOFF[c]:MOFF[c] + w],
                        )
                        pts[(c, half)] = pt
                return pts

            def av_block(hp, pts):
                yt_pair = []
                for half in range(2):
                    h = hp * 2 + half
                    yt = psyt.tile([65, 512], f32, tag="yt",
                                   name=f"yt{hp}_{half}")
                    for ci, c in enumerate(AVORD):
                        w, off = W_C[c], OFF_C[c]
                        nc.tensor.matmul(
                            yt[:, off:off + w],
                            v_sb[c][:, h * 65:(h + 1) * 65],
                            pts[(c, half)][:, 0:w],
                            start=(ci == 0), stop=(ci == 6),
                        )
                    yt_pair.append(yt)
                ytu = ytsp.tile([128, CH], f32r, tag=f"ytu{hp}",
                                name=f"ytu{hp}")
                nc.scalar.copy(ytu[0:64, :], yt_pair[0][0:64, :])
                nc.scalar.copy(ytu[64:128, :], yt_pair[1][0:64, :])
                for half in range(2):
                    dt_ = smp.tile([1, 512], f32, tag="dt",
                                   name=f"dt{hp}_{half}")
                    nc.scalar.copy(dt_[:], yt_pair[half][64:65, :])
                    nc.sync.dma_start(
                        d16[2 * hp + half:2 * hp + half + 1, :], dt_[:])
                return ytu

            # ---------- software-pipelined head-pair loop ----------
            yts = []
            qk_state = qkv_rope(0)
            for hp in range(8):
                pts = sc_block(hp, *qk_state)
                if hp < 7:
                    qk_state = qkv_rope(hp + 1)
                yts.append(av_block(hp, pts))

            # ---------- batched normalization (in place, rounds to f32r) ----
            r16 = smp.tile([16, 512], f32r, tag="r16")
            with nc.allow_low_precision(reason="f32r recip for PE broadcast"):
                nc.vector.reciprocal(r16[:], d16[:])
            for hp in range(8):
                prb = psmm.tile([128, 512], f32, tag="mm", name=f"prb{hp}")
                nc.tensor.matmul(prb[:], trsel[:, hp * 128:(hp + 1) * 128],
                                 r16[:], start=True, stop=True)
                nc.vector.tensor_mul(yts[hp][0:64, :], yts[hp][0:64, :],
                                     prb[0:64, :])
                nc.vector.tensor_mul(yts[hp][64:128, :], yts[hp][64:128, :],
                                     prb[64:128, :])

            # ---------- projection (transposed output) ----------
            for cc in range(8):
                wp_sl = wsl.tile([128, 1024], f32r, tag="wslab",
                                 name=f"wp{cc}")
                nc.sync.dma_start(wp_sl[:], wps[cc * 128:(cc + 1) * 128, :])
                po = psmm.tile([128, 512], f32, tag="mm", name=f"po{cc}")
                for hp in range(8):
                    nc.tensor.matmul(
                        po[:], wp_sl[:, hp * 128:(hp + 1) * 128], yts[hp][:],
                        start=(hp == 0), stop=(hp == 7),
                    )
                osb = big.tile([128, 512], f32, tag="big", name=f"osb{cc}")
                nc.scalar.copy(osb[:], po[:])
                nc.sync.dma_start(outT[cc * 128:(cc + 1) * 128, :], osb[:])

    nc.compile()
    return nc


def _host_inputs(x, w_attn, w_proj):
    """Build the 8 per-core input maps."""
    inv_freq = 1.0 / (10000.0 ** (np.arange(0, HD, 2, dtype=np.float32) / HD))
    iff = np.concatenate([inv_freq, inv_freq])  # [64]

    def cos_sin(pos):
        ang = pos[None, :].astype(np.float32) * iff[:, None]
        c = np.concatenate([np.cos(ang), np.cos(ang)], 0).astype(np.float32)
        s = np.concatenate([np.sin(ang), np.sin(ang)], 0).astype(np.float32)
        return np.ascontiguousarray(c), np.ascontiguousarray(s)

    P2 = np.zeros((128, 128), np.float32)
    for blk in range(2):
        o = blk * 64
        for d in range(32):
            P2[o + d + 32, o + d] = -1.0
            P2[o + d, o + d + 32] = 1.0

    rsel = np.zeros((16, C), np.float32)
    for h in range(16):
        hp, half = h // 2, h % 2
        rsel[h, hp * 128 + half * 64: hp * 128 + half * 64 + 64] = 1.0
    ones16 = np.ones((128, 16), np.float32)

    def shuffle_lhsT(w):
        # rows (kc*128 + c_lo), cols (hp*128 + d) ->
        # rows (hp*128 + c_lo), cols (kc*128 + d)
        return np.ascontiguousarray(
            w.reshape(8, 128, 8, 128).transpose(2, 1, 0, 3).reshape(C, C)
        )

    wq = shuffle_lhsT(w_attn[:, 0:C])
    wk = shuffle_lhsT(w_attn[:, C:2 * C])
    wvm = np.ascontiguousarray(w_attn[:, 2 * C:3 * C])
    wp = shuffle_lhsT(w_proj)

    in_maps = []
    for core in range(NCORES):
        b, j = core // 4, core % 4
        q0 = j * CH
        kv_gk = np.full(KV, -1, np.int64)
        kv_gk[0:512] = q0 + np.arange(CH)
        halo = q0 - 256 + np.arange(256)
        kv_gk[512:768] = np.where(halo >= 0, halo, -1)
        kv_gk[768:772] = np.arange(4)

        xTc = np.zeros((C, KV), np.float32)
        valid = kv_gk >= 0
        xTc[:, valid] = x[b, kv_gk[valid]].T

        cq, sq = cos_sin(q0 + np.arange(CH))
        ck, sk = cos_sin(np.maximum(kv_gk, 0))

        gq = q0 + np.arange(CH)
        mask = np.zeros((128, MTOT), np.float32)
        for c in range(7):
            rows = c * 128 + np.arange(128)
            gk = kv_gk[rows]
            qw = gq[OFF_C[c]:OFF_C[c] + W_C[c]]
            real = (rows < 772) & (gk >= 0)
            g = np.where(real, gk, 0)[:, None]
            qq = qw[None, :]
            is_sink = ((rows >= 768) & (rows < 772))[:, None]
            allow = np.where(
                is_sink,
                (g <= qq) & (qq - g >= WIN),
                (g <= qq) & (qq - g < WIN),
            )
            allow &= real[:, None]
            mask[:, MOFF[c]:MOFF[c] + W_C[c]] = allow.astype(np.float32)

        in_maps.append({
            "xT": xTc, "wqs": wq, "wks": wk, "wv": wvm, "wps": wp,
            "cos_q": cq, "sin_q": sq, "cos_k": ck, "sin_k": sk,
            "masks": mask, "p2": P2, "rsel": rsel, "ones": ones16,
        })
    return in_maps


def kernel(x, w_attn, w_proj):
    from concourse import bass_utils

    x = np.asarray(x, np.float32)
    w_attn = np.asarray(w_attn, np.float32)
    w_proj = np.asarray(w_proj, np.float32)

    if "nc" not in _cache:
        _cache["nc"] = _build_nc()
    nc = _cache["nc"]

    in_maps = _host_inputs(x, w_attn, w_proj)
    res = bass_utils.run_bass_kernel_spmd(nc, in_maps, list(range(NCORES)),
                                          **_cache.get("run_kwargs", {}))
    _cache["last_result"] = res

    y = np.zeros((B, T, C), np.float32)
    for core in range(NCORES):
        b, j = core // 4, core % 4
        y[b, j * CH:(j + 1) * CH, :] = res.results[core]["outT"].T
    return y
